# revision 1
# baseline (speedup 1.0000x reference)
"""LRU (complex diagonal linear recurrence, fwd+bwd) on 8 TRN2 NeuronCores — v3.

Sequence-parallel over T (TC=2048/core). Per core, per state-tile nt:
  BuT = B_norm @ x^T                                     (PE, fp16 matmuls)
  pre-rotation shared between directions (symmetry trick, packed [re|im]):
    prod1 = [cos|cos] (.) [bu_re|bu_im]   prod2 = [sin|sin] (.) [bu_im|bu_re]
    wf_re = p1_lo + p2_lo   wf_im = p1_hi - p2_hi     (fwd scan input)
    qr_re = rev(p1_lo - p2_lo)  qr_im = rev(p1_hi + p2_hi)  (bwd, reversed)
  4 real first-order scans with multiplier r (DVE; Pool/GpSimd cannot run
  elementwise/scan ops in this NEFF lowering -- compile-time ISA check)
  carry exchange: one 8KB AllGather of raw chunk-end scan states, read
  back with a single contiguous gather (64B runs; the naive per-nt strided
  gathers cost 16x more DMA descriptors); all phase constants fold into
  host-precomputed packed weights W4; two TT + two segmented reduces per
  nt recover the complex carries for both directions.
  correction: v += rpow (.) chv, in place (Act computes the product with a
  per-partition scale, DVE adds)
  post-rotation STRIPED in 1024-column blocks so output-projection PSUM
  chains (FD=512 each; one matmul must fit one PSUM bank) complete within
  one stripe -- no end-of-kernel PE tail:
    s_f = e^{+i theta tau} v2_f     (in place over v)
    s_b = e^{-i theta (TC-1-rho)} v2_b  (into s_b tiles, written reversed)
  y^T = C-projections (fp16 matmuls, PE).  D (.) x added on HOST.
"""

import numpy as np
from contextlib import ExitStack

import concourse.bass as bass
import concourse.tile as tile
from concourse import bacc, mybir
from concourse.bass_utils import run_bass_kernel_spmd

NCORES = 8
T, N, H = 16384, 512, 512
TC = T // NCORES          # 2048 timesteps per core
NT = N // 128             # 4 partition tiles of the state dim
HT = H // 128             # 4 partition tiles of the channel dim
KH = H // 128             # contraction subtiles for Bu matmul
LB = 1024                 # output stripe width
NL = TC // LB
F16 = mybir.dt.float16
F32 = mybir.dt.float32
MUL = mybir.AluOpType.mult
ADD = mybir.AluOpType.add
SUB = mybir.AluOpType.subtract
COPY = mybir.ActivationFunctionType.Copy

_CACHE = {}


def _dup2(ap_2d, w):
    """(128, w) AP -> (128, 2, w) with the row repeated twice (step-0 dup)."""
    return ap_2d.rearrange("p (c t) -> p c t", c=1).broadcast_to([128, 2, w])


def _build_nc(profile=False, iters=1, fake_cc=False, percc=False):
    nc = bacc.Bacc(
        "TRN2", target_bir_lowering=False, debug=False,
        enable_asserts=False, num_devices=1 if profile else NCORES,
    )
    di = lambda n, s, d=F32: nc.dram_tensor(n, s, d, kind="ExternalInput")
    xT_d = di("xT", [H, TC], F16)
    BTre_d = di("BTre", [H, N], F16)
    BTim_d = di("BTim", [H, N], F16)
    cos_d = di("cosT", [N, TC], F16)
    sin_d = di("sinT", [N, TC], F16)
    rpw_d = di("rpow", [N, TC], F16)
    cst_d = di("consts", [N, 8])            # col0 = r
    CT_d = {(d_, c_): di(f"CT{d_}{c_}", [N, H], F16)
            for d_ in "fb" for c_ in "ri"}
    W4_d = di("W4", [N, 64])
    yT_d = nc.dram_tensor("yT", [H, TC], F32, kind="ExternalOutput")
    bin_d = nc.dram_tensor("ccin", [128, 16], F32)
    bout_d = nc.dram_tensor("ccout", [NCORES, 128, 16], F32)
    bin4_d = [nc.dram_tensor(f"ccin{i}", [128, 4], F32) for i in range(NT)] if percc else None
    bout4_d = [nc.dram_tensor(f"ccout{i}", [NCORES, 128, 4], F32) for i in range(NT)] if percc else None

    with tile.TileContext(nc) as tc, ExitStack() as ctx:
        pool = lambda name, bufs: ctx.enter_context(tc.tile_pool(name=name, bufs=bufs))
        p_xT = pool("xT", 4)
        p_b4 = pool("b4", 4)
        p_BT = pool("BT", 8)
        p_CT = pool("CT", 16)
        p_cos = pool("cos", 4)
        p_sin = pool("sin", 4)
        p_rpw = pool("rpw", 2)
        p_cst = pool("cst", 4)
        p_bu = pool("bu", 1)
        p_wq = pool("wq", 4)
        p_v = pool("v", 8)
        p_yo = pool("yo", 2)
        p_sm = pool("sm", 2)
        p_ch = pool("ch", 4)
        p_bups = ctx.enter_context(tc.tile_pool(name="bups", bufs=2, space="PSUM"))
        p_ops = ctx.enter_context(tc.tile_pool(name="ops", bufs=2, space="PSUM"))

        for _iter in range(iters):
            # ---- loads: first tables for nt0/nt1, then matmul operands ----
            xT_sb = []
            for h in range(HT):
                t_ = p_xT.tile([128, TC], F16, tag="xT", name="xt")
                nc.sync.dma_start(t_[:], xT_d[h * 128:(h + 1) * 128, :])
                xT_sb.append(t_)
            BT_sb = {}
            for nm, dd in (("re", BTre_d), ("im", BTim_d)):
                for h in range(HT):
                    t_ = p_BT.tile([128, N], F16, tag="BT", name="bt")
                    nc.sync.dma_start(t_[:], dd[h * 128:(h + 1) * 128, :])
                    BT_sb[(nm, h)] = t_
            tabs1 = {}
            for nt in range(2):
                nsl = slice(nt * 128, (nt + 1) * 128)
                cos_t = p_cos.tile([128, TC], F16, tag="cos", name="c0")
                nc.sync.dma_start(cos_t[:], cos_d[nsl, :])
                sin_t = p_sin.tile([128, TC], F16, tag="sin", name="s0")
                nc.sync.dma_start(sin_t[:], sin_d[nsl, :])
                tabs1[nt] = (cos_t, sin_t)
            cst_sb = []
            for nt in range(NT):
                t_ = p_cst.tile([128, 8], F32, tag="cst", name="cs")
                nc.sync.dma_start(t_[:], cst_d[nt * 128:(nt + 1) * 128, :])
                cst_sb.append(t_)

            # ---- phase 1: Bu matmuls, shared pre-rotations, scans ----
            v_sb = {}
            chv_sb = {}
            epk = p_sm.tile([128, 16], F32, tag="epk", name="ep")      # (nt, dir) -> packed (128, 2TC) f16 scan outputs
            for nt in range(NT):
                nsl = slice(nt * 128, (nt + 1) * 128)
                if nt in tabs1:
                    cos_t, sin_t = tabs1[nt]
                else:
                    cos_t = p_cos.tile([128, TC], F16, tag="cos", name="c1")
                    nc.sync.dma_start(cos_t[:], cos_d[nsl, :])
                    sin_t = p_sin.tile([128, TC], F16, tag="sin", name="s1")
                    nc.sync.dma_start(sin_t[:], sin_d[nsl, :])
                    tabs1[nt] = (cos_t, sin_t)
                epk4 = (p_ch.tile([128, 4], F32, tag="ek4", name="e4")
                        if percc else None)
                bu = p_bu.tile([128, 2 * TC], F16, tag="bu", name="bu")
                for ci, nm in enumerate(("re", "im")):
                    for half in range(2):
                        ps = p_bups.tile([128, TC // 2], F32, tag="bups", name="ps")
                        for lc in range(2):
                            sl = slice(half * 1024 + lc * 512, half * 1024 + (lc + 1) * 512)
                            psl = slice(lc * 512, (lc + 1) * 512)
                            for kh in range(KH):
                                nc.tensor.matmul(
                                    ps[:, psl],
                                    BT_sb[(nm, kh)][:, nsl],
                                    xT_sb[kh][:, sl],
                                    start=(kh == 0), stop=(kh == KH - 1),
                                )
                        nc.scalar.copy(
                            bu[:, ci * TC + half * 1024: ci * TC + (half + 1) * 1024],
                            ps[:])
                bu3 = bu[:].rearrange("p (c t) -> p c t", c=2)
                bsw = bu3[:, ::-1, :]
                prod1 = p_wq.tile([128, 2 * TC], F16, tag="wq", name="p1")
                qr = p_wq.tile([128, 2 * TC], F16, tag="wq", name="qr")
                p13 = prod1[:].rearrange("p (c t) -> p c t", c=2)
                p2l = p_b4.tile([128, TC], F16, tag="b4", name="2l")
                p2h = p_b4.tile([128, TC], F16, tag="b4", name="2h")
                nc.vector.tensor_tensor(p13, _dup2(cos_t[:], TC), bu3, MUL)
                nc.vector.tensor_tensor(p2l[:], sin_t[:], bu[:, TC:2 * TC], MUL)
                nc.vector.tensor_tensor(p2h[:], sin_t[:], bu[:, 0:TC], MUL)
                # qr = bwd scan input, stored time-reversed
                nc.vector.tensor_tensor(qr[:, 0:TC][:, ::-1],
                                        prod1[:, 0:TC], p2l[:], SUB)
                nc.vector.tensor_tensor(qr[:, TC:2 * TC][:, ::-1],
                                        prod1[:, TC:2 * TC], p2h[:], ADD)
                # wf in place over prod1
                nc.vector.tensor_tensor(prod1[:, 0:TC],
                                        prod1[:, 0:TC], p2l[:], ADD)
                nc.vector.tensor_tensor(prod1[:, TC:2 * TC],
                                        prod1[:, TC:2 * TC], p2h[:], SUB)
                wf = prod1
                rbc = cst_sb[nt][:, 0:1].broadcast_to([128, TC])
                for di_, (d_, src) in enumerate((("f", wf), ("b", qr))):
                    v = p_v.tile([128, 2 * TC], F16, tag="v", name="v")
                    nc.vector.tensor_tensor_scan(
                        v[:, 0:TC], rbc, src[:, 0:TC], 0.0, MUL, ADD)
                    nc.vector.tensor_tensor_scan(
                        v[:, TC:2 * TC], rbc, src[:, TC:2 * TC], 0.0, MUL, ADD)
                    v_sb[(nt, d_)] = v
                    if percc:
                        nc.scalar.copy(epk4[:, 2 * di_:2 * di_ + 2],
                                       v[:, TC - 1::TC])
                    else:
                        nc.scalar.copy(
                            epk[:, (0 if d_ == "f" else 8) + nt * 2:
                                (0 if d_ == "f" else 8) + nt * 2 + 2],
                            v[:, TC - 1::TC])
                if percc:
                    # fire this tile's exchange while later tiles still scan
                    nc.sync.dma_start(bin4_d[nt][:, :], epk4[:])
                    if profile or fake_cc:
                        for j in range(NCORES):
                            nc.sync.dma_start(bout4_d[nt].ap()[j, :, :],
                                              bin4_d[nt][:, :])
                    else:
                        nc.gpsimd.collective_compute(
                            "AllGather", mybir.AluOpType.bypass,
                            replica_groups=[list(range(NCORES))],
                            ins=[bin4_d[nt].ap().opt()],
                            outs=[bout4_d[nt].ap().opt()],
                        )

            if not percc:
                # ---- single carry exchange of all raw end states ----
                nc.sync.dma_start(bin_d[:, :], epk[:])
                if profile or fake_cc:
                    for j in range(NCORES):
                        nc.sync.dma_start(bout_d.ap()[j, :, :], bin_d[:, :])
                else:
                    nc.gpsimd.collective_compute(
                        "AllGather", mybir.AluOpType.bypass,
                        replica_groups=[list(range(NCORES))],
                        ins=[bin_d.ap().opt()], outs=[bout_d.ap().opt()],
                    )
            # one contiguous gather of all end states: 64B runs instead of
            # 16 strided DMAs of 8B runs (descriptor-count bound)
            egall = p_sm.tile([128, 128], F32, tag="ega", name="ga")
            if percc:
                for nt in range(NT):
                    nc.sync.dma_start(
                        egall[:, nt * 32:(nt + 1) * 32].rearrange("p (j q) -> p j q", j=NCORES),
                        bout4_d[nt].ap().rearrange("j p q -> p j q"))
            else:
                nc.sync.dma_start(
                    egall[:].rearrange("p (j q) -> p j q", j=NCORES),
                    bout_d.ap().rearrange("j p q -> p j q"))
            for nt in range(NT):
                nsl = slice(nt * 128, (nt + 1) * 128)
                w4_t = p_sm.tile([128, 64], F32, tag="w4", name="w4")
                nc.sync.dma_start(w4_t[:], W4_d[nsl, :])
                if percc:
                    e4 = egall[:, nt * 32:(nt + 1) * 32].rearrange(
                        "p (j g c) -> p j g c", j=NCORES, g=2)
                else:
                    e4 = egall[:].rearrange(
                        "p (j g x) -> p j g x", j=NCORES, g=2)[:, :, :, nt * 2:nt * 2 + 2]
                chv = p_ch.tile([128, 4], F32, tag="chv", name="ch")
                for half, csl in ((0, slice(0, 4, 2)), (1, slice(1, 4, 2))):
                    w4v = w4_t[:, half * 32:(half + 1) * 32].rearrange(
                        "p (j g c) -> p j g c", j=NCORES, g=2)
                    pr = p_sm.tile([128, 32], F32, tag="pr", name="pr")
                    nc.vector.tensor_tensor(
                        pr[:].rearrange("p (g j c) -> p j g c", g=2, j=NCORES),
                        w4v, e4, MUL)
                    nc.vector.tensor_reduce(
                        chv[:, csl].rearrange("p (s o) -> p s o", o=1),
                        pr[:].rearrange("p (s m) -> p s m", s=2),
                        mybir.AxisListType.X, ADD)
                chv_sb[nt] = chv

            # prefetch correction tables while phase 1 drains
            # prefetch correction tables while phase 1 drains
            rpw_sb = {}
            for nt in range(2):
                rpw_t = p_rpw.tile([128, TC], F16, tag="rpw", name="rq")
                nc.sync.dma_start(rpw_t[:], rpw_d[nt * 128:(nt + 1) * 128, :])
                rpw_sb[nt] = rpw_t

            # ---- phase 2a: per nt full-width corrections ----
            for nt in range(NT):
                nsl = slice(nt * 128, (nt + 1) * 128)
                chv = chv_sb[nt]
                if nt in rpw_sb:
                    rpw_t = rpw_sb[nt]
                else:
                    rpw_t = p_rpw.tile([128, TC], F16, tag="rpw", name="rp")
                    nc.sync.dma_start(rpw_t[:], rpw_d[nsl, :])
                v = v_sb[(nt, "f")]
                vb = v_sb[(nt, "b")]
                for ci, (vv, sc) in enumerate(((v, 0), (v, 1), (vb, 2), (vb, 3))):
                    t_ = p_b4.tile([128, TC], F16, tag="b4", name="t4")
                    nc.scalar.activation(t_[:], rpw_t[:], COPY,
                                         scale=chv[:, sc:sc + 1])
                    half = slice(0, TC) if ci % 2 == 0 else slice(TC, 2 * TC)
                    nc.vector.tensor_tensor(vv[:, half], t_[:], vv[:, half], ADD)

            # ---- late loads (needed only from mid-kernel on) ----
            CT_sb = {}
            for key, dd in CT_d.items():
                for nt in range(NT):
                    t_ = p_CT.tile([128, H], F16, tag="CT", name="ct")
                    nc.sync.dma_start(t_[:], dd[nt * 128:(nt + 1) * 128, :])
                    CT_sb[key + (nt,)] = t_
            rpw_sb = {}

            # ---- phase 2b: lc-major striped post-rot + output projections ----
            sb_sb = {nt: p_wq.tile([128, 2 * TC], F16, tag="wq", name="sb")
                     for nt in range(NT)}
            for lc in range(NL):
                a, b = lc * LB, (lc + 1) * LB
                ra, rb = TC - b, TC - a
                lsl = slice(a, b)
                hsl_i = slice(TC + a, TC + b)
                rsl = slice(ra, rb)
                for nt in range(NT):
                    cos_t, sin_t = tabs1[nt]
                    v = v_sb[(nt, "f")]
                    vb = v_sb[(nt, "b")]
                    sb = sb_sb[nt]
                    v3 = v[:].rearrange("p (c t) -> p c t", c=2)
                    vsw = v3[:, ::-1, :]
                    vb3 = vb[:].rearrange("p (c t) -> p c t", c=2)
                    vbsw = vb3[:, ::-1, :]
                    pp = p_b4.tile([128, 2 * LB], F16, tag="b4", name="pp")
                    zz = p_b4.tile([128, 2 * LB], F16, tag="b4", name="zz")
                    pp3 = pp[:].rearrange("p (c t) -> p c t", c=2)
                    zz3 = zz[:].rearrange("p (c t) -> p c t", c=2)
                    nc.vector.tensor_tensor(
                        pp3, _dup2(cos_t[:], TC)[:, :, lsl], v3[:, :, lsl], MUL)
                    nc.vector.tensor_tensor(
                        zz3, _dup2(sin_t[:], TC)[:, :, lsl], vsw[:, :, lsl], MUL)
                    nc.vector.tensor_tensor(v[:, lsl], pp[:, 0:LB], zz[:, 0:LB], SUB)
                    nc.vector.tensor_tensor(v[:, hsl_i], pp[:, LB:2 * LB],
                                            zz[:, LB:2 * LB], ADD)
                    # bwd products on the (otherwise idle) Pool engine
                    ppb = p_b4.tile([128, 2 * LB], F16, tag="b4", name="pb")
                    zzb = p_b4.tile([128, 2 * LB], F16, tag="b4", name="zb")
                    ppb3 = ppb[:].rearrange("p (c t) -> p c t", c=2)
                    zzb3 = zzb[:].rearrange("p (c t) -> p c t", c=2)
                    nc.vector.tensor_tensor(
                        ppb3, _dup2(cos_t[:, ::-1], TC)[:, :, rsl], vb3[:, :, rsl], MUL)
                    nc.vector.tensor_tensor(
                        zzb3, _dup2(sin_t[:, ::-1], TC)[:, :, rsl], vbsw[:, :, rsl], MUL)
                    nc.vector.tensor_tensor(sb[:, lsl][:, ::-1],
                                            ppb[:, 0:LB], zzb[:, 0:LB], ADD)
                    nc.vector.tensor_tensor(sb[:, hsl_i][:, ::-1],
                                            ppb[:, LB:2 * LB], zzb[:, LB:2 * LB], SUB)
                for ht in range(HT):
                    hsl = slice(ht * 128, (ht + 1) * 128)
                    # FD=512 sub-chains: one matmul must fit one PSUM bank
                    for half in range(2):
                        aa = a + half * 512
                        l5 = slice(aa, aa + 512)
                        h5 = slice(TC + aa, TC + aa + 512)
                        ps = p_ops.tile([128, 512], F32, tag="ops", name="op")
                        groups = []
                        for nt in range(NT):
                            groups.append((CT_sb[("f", "r", nt)], v_sb[(nt, "f")], l5))
                            groups.append((CT_sb[("f", "i", nt)], v_sb[(nt, "f")], h5))
                            groups.append((CT_sb[("b", "r", nt)], sb_sb[nt], l5))
                            groups.append((CT_sb[("b", "i", nt)], sb_sb[nt], h5))
                        for gi, (ct, sv, sl_) in enumerate(groups):
                            nc.tensor.matmul(
                                ps[:], ct[:, hsl], sv[:, sl_],
                                start=(gi == 0), stop=(gi == len(groups) - 1),
                            )
                        yo = p_yo.tile([128, 512], F32, tag="yo", name="yo")
                        nc.scalar.copy(yo[:], ps[:])
                        nc.sync.dma_start(yT_d[hsl, l5], yo[:])

    nc.compile()
    return nc


def _host_prep(x, theta_log, nu_log, B_re, B_im, C_re, C_im, C_re2, C_im2, D):
    f64 = np.float64
    theta = np.exp(theta_log.astype(f64))
    r = np.exp(-np.exp(nu_log.astype(f64)))
    gamma = np.sqrt(1.0 - r ** 2)
    Bn = (B_re.astype(f64) + 1j * B_im.astype(f64)) * gamma[:, None]
    Lam = r * np.exp(1j * theta)
    tau = np.arange(TC, dtype=f64)
    cosT = np.cos(theta[:, None] * tau).astype(np.float16)
    sinT = np.sin(theta[:, None] * tau).astype(np.float16)
    rpow = (r[:, None] ** (tau + 1)).astype(np.float16)
    consts = np.zeros((N, 8), np.float32)
    consts[:, 0] = r
    xT = np.ascontiguousarray(x.T.astype(np.float16))        # (H, T)
    BTre = np.ascontiguousarray(Bn.real.T.astype(np.float16))
    BTim = np.ascontiguousarray(Bn.imag.T.astype(np.float16))
    C1 = C_re.astype(f64) + 1j * C_im.astype(f64)
    C2 = C_re2.astype(f64) + 1j * C_im2.astype(f64)
    CT = {
        ("f", "r"): C1.real.T, ("f", "i"): -C1.imag.T,
        ("b", "r"): C2.real.T, ("b", "i"): -C2.imag.T,
    }
    CT = {k: np.ascontiguousarray(v.astype(np.float16)) for k, v in CT.items()}
    LamTC = Lam ** TC
    phase = np.exp(1j * theta * TC)
    W4 = []
    for k in range(NCORES):
        wf = np.zeros((N, NCORES), np.complex128)
        wb = np.zeros((N, NCORES), np.complex128)
        for j in range(k):
            wf[:, j] = phase * LamTC ** (k - 1 - j)
        for j in range(k + 1, NCORES):
            wb[:, j] = phase * LamTC ** (j - k - 1)

        # layout matches the contiguous gather: element (j, g, c) of the
        # gathered end states pairs with W4re/W4im at flat index j*4+g*2+c
        w4re = np.zeros((N, NCORES, 2, 2), np.float64)
        w4im = np.zeros((N, NCORES, 2, 2), np.float64)
        w4re[:, :, 0, 0] = wf.real
        w4re[:, :, 0, 1] = -wf.imag
        w4re[:, :, 1, 0] = wb.real
        w4re[:, :, 1, 1] = -wb.imag
        w4im[:, :, 0, 0] = wf.imag
        w4im[:, :, 0, 1] = wf.real
        w4im[:, :, 1, 0] = wb.imag
        w4im[:, :, 1, 1] = wb.real
        w4 = np.concatenate(
            [w4re.reshape(N, 32), w4im.reshape(N, 32)], axis=1).astype(np.float32)
        W4.append(np.ascontiguousarray(w4))
    Dx = (D.astype(f64)[None, :] * x.astype(f64)).astype(np.float32)
    return xT, BTre, BTim, cosT, sinT, rpow, consts, CT, W4, Dx


def make_in_maps(inputs):
    xT, BTre, BTim, cosT, sinT, rpow, consts, CT, W4, Dx = _host_prep(**inputs)
    in_maps = []
    for k in range(NCORES):
        in_maps.append({
            "xT": np.ascontiguousarray(xT[:, k * TC:(k + 1) * TC]),
            "BTre": BTre, "BTim": BTim,
            "cosT": cosT, "sinT": sinT, "rpow": rpow, "consts": consts,
            "CTfr": CT[("f", "r")], "CTfi": CT[("f", "i")],
            "CTbr": CT[("b", "r")], "CTbi": CT[("b", "i")],
            "W4": W4[k],
        })
    return in_maps, Dx


def kernel(**inputs):
    if "nc" not in _CACHE:
        _CACHE["nc"] = _build_nc()
    nc = _CACHE["nc"]
    in_maps, Dx = make_in_maps(inputs)
    res = run_bass_kernel_spmd(nc, in_maps, core_ids=list(range(NCORES)))
    yT = np.concatenate([res.results[k]["yT"] for k in range(NCORES)], axis=1)
    return (np.ascontiguousarray(yT.T) + Dx).astype(np.float32)



# revision 20
# speedup vs baseline: 4.1906x; 4.1906x over previous
"""LRU (complex diagonal linear recurrence, fwd+bwd) on 8 TRN2 NeuronCores — v3.

Sequence-parallel over T (TC=2048/core). Per core, per state-tile nt:
  BuT = B_norm @ x^T                                     (PE, fp16 matmuls)
  pre-rotation shared between directions (symmetry trick, packed [re|im]):
    prod1 = [cos|cos] (.) [bu_re|bu_im]   prod2 = [sin|sin] (.) [bu_im|bu_re]
    wf_re = p1_lo + p2_lo   wf_im = p1_hi - p2_hi     (fwd scan input)
    qr_re = rev(p1_lo - p2_lo)  qr_im = rev(p1_hi + p2_hi)  (bwd, reversed)
  4 real first-order scans with multiplier r (DVE; Pool/GpSimd cannot run
  elementwise/scan ops in this NEFF lowering -- compile-time ISA check)
  carry exchange: one 8KB AllGather of raw chunk-end scan states, read
  back with a single contiguous gather (64B runs; the naive per-nt strided
  gathers cost 16x more DMA descriptors); all phase constants fold into
  host-precomputed packed weights W4; two TT + two segmented reduces per
  nt recover the complex carries for both directions.
  correction: v += rpow (.) chv, in place (Act computes the product with a
  per-partition scale, DVE adds)
  post-rotation STRIPED in 1024-column blocks so output-projection PSUM
  chains (FD=512 each; one matmul must fit one PSUM bank) complete within
  one stripe -- no end-of-kernel PE tail:
    s_f = e^{+i theta tau} v2_f     (in place over v)
    s_b = e^{-i theta (TC-1-rho)} v2_b  (into s_b tiles, written reversed)
  y^T = C-projections (fp16 matmuls, PE).  D (.) x added on HOST.
"""

import numpy as np
from contextlib import ExitStack

import concourse.bass as bass
import concourse.tile as tile
from concourse import bacc, mybir
from concourse.bass_utils import run_bass_kernel_spmd

NCORES = 8
T, N, H = 16384, 512, 512
TC = T // NCORES          # 2048 timesteps per core
NT = N // 128             # 4 partition tiles of the state dim
HT = H // 128             # 4 partition tiles of the channel dim
KH = H // 128             # contraction subtiles for Bu matmul
LB = 1024                 # output stripe width
NL = TC // LB
F16 = mybir.dt.float16
F32 = mybir.dt.float32
MUL = mybir.AluOpType.mult
ADD = mybir.AluOpType.add
SUB = mybir.AluOpType.subtract
COPY = mybir.ActivationFunctionType.Copy

_CACHE = {}


def _dup2(ap_2d, w):
    """(128, w) AP -> (128, 2, w) with the row repeated twice (step-0 dup)."""
    return ap_2d.rearrange("p (c t) -> p c t", c=1).broadcast_to([128, 2, w])


def _build_nc(profile=False, iters=1, fake_cc=False, percc=False,
              corr_pool=False, pre_pool=False, dots_pool=False):
    nc = bacc.Bacc(
        "TRN2", target_bir_lowering=False, debug=False,
        enable_asserts=False, num_devices=1 if profile else NCORES,
    )
    di = lambda n, s, d=F32: nc.dram_tensor(n, s, d, kind="ExternalInput")
    xT_d = di("xT", [H, TC], F16)
    BTre_d = di("BTre", [H, N], F16)
    BTim_d = di("BTim", [H, N], F16)
    cos_d = di("cosT", [N, TC], F16)
    sin_d = di("sinT", [N, TC], F16)
    rpw_d = di("rpow", [N, TC], F16)
    cst_d = di("consts", [N, 8])            # col0 = r
    CT_d = {(d_, c_): di(f"CT{d_}{c_}", [N, H], F16)
            for d_ in "fb" for c_ in "ri"}
    W4_d = di("W4", [N, 64])
    yT_d = nc.dram_tensor("yT", [H, TC], F32, kind="ExternalOutput")
    # double-buffer the exchange tensors so iteration i+1's exchange does
    # not serialize behind iteration i's consumers
    NB = 2
    bin_db = [nc.dram_tensor(f"ccin_{j}", [128, 16], F32) for j in range(NB)]
    bout_db = [nc.dram_tensor(f"ccout_{j}", [NCORES, 128, 16], F32)
               for j in range(NB)]
    bin4_db = [[nc.dram_tensor(f"ccin{i}_{j}", [128, 4], F32)
                for i in range(NT)] for j in range(NB)] if percc else None
    bout4_db = [[nc.dram_tensor(f"ccout{i}_{j}", [NCORES, 128, 4], F32)
                 for i in range(NT)] for j in range(NB)] if percc else None

    with tile.TileContext(nc) as tc, ExitStack() as ctx:
        pool = lambda name, bufs: ctx.enter_context(tc.tile_pool(name=name, bufs=bufs))
        p_xT = pool("xT", 4)
        p_b4 = pool("b4", 4)
        p_BT = pool("BT", 8)
        p_CT = pool("CT", 16)
        p_cos = pool("cos", 4)
        p_sin = pool("sin", 4)
        p_rpw = pool("rpw", 2)
        p_cst = pool("cst", 4)
        p_bu = pool("bu", 1)
        p_wq = pool("wq", 4)
        p_v = pool("v", 8)
        p_yo = pool("yo", 2)
        p_sm = pool("sm", 2)
        p_ch = pool("ch", 4)
        p_bups = ctx.enter_context(tc.tile_pool(name="bups", bufs=2, space="PSUM"))
        p_ops = ctx.enter_context(tc.tile_pool(name="ops", bufs=4, space="PSUM"))

        def emit_loads(it):
            """Input loads for one iteration: xT, BT, cos/sin for nt0/1, cst."""
            xT_sb = []
            for h in range(HT):
                t_ = p_xT.tile([128, TC], F16, tag="xT", name="xt")
                nc.sync.dma_start(t_[:], xT_d[h * 128:(h + 1) * 128, :])
                xT_sb.append(t_)
            BT_sb = {}
            for nm, dd in (("re", BTre_d), ("im", BTim_d)):
                for h in range(HT):
                    t_ = p_BT.tile([128, N], F16, tag="BT", name="bt")
                    nc.sync.dma_start(t_[:], dd[h * 128:(h + 1) * 128, :])
                    BT_sb[(nm, h)] = t_
            tabs1 = {}
            for nt in range(2):
                nsl = slice(nt * 128, (nt + 1) * 128)
                cos_t = p_cos.tile([128, TC], F16, tag="cos", name="c0")
                nc.sync.dma_start(cos_t[:], cos_d[nsl, :])
                sin_t = p_sin.tile([128, TC], F16, tag="sin", name="s0")
                nc.sync.dma_start(sin_t[:], sin_d[nsl, :])
                tabs1[nt] = (cos_t, sin_t)
            cst_sb = []
            for nt in range(NT):
                t_ = p_cst.tile([128, 8], F32, tag="cst", name="cs")
                nc.sync.dma_start(t_[:], cst_d[nt * 128:(nt + 1) * 128, :])
                cst_sb.append(t_)
            return dict(xT=xT_sb, BT=BT_sb, tabs=tabs1, cst=cst_sb)

        def emit_bu(front, nt):
            """Bu matmuls + PSUM->SBUF copies for one state tile."""
            nsl = slice(nt * 128, (nt + 1) * 128)
            bu = p_bu.tile([128, 2 * TC], F16, tag="bu", name="bu")
            for ci, nm in enumerate(("re", "im")):
                for half in range(2):
                    ps = p_bups.tile([128, TC // 2], F32, tag="bups", name="ps")
                    for lc in range(2):
                        sl = slice(half * 1024 + lc * 512, half * 1024 + (lc + 1) * 512)
                        psl = slice(lc * 512, (lc + 1) * 512)
                        for kh in range(KH):
                            nc.tensor.matmul(
                                ps[:, psl],
                                front["BT"][(nm, kh)][:, nsl],
                                front["xT"][kh][:, sl],
                                start=(kh == 0), stop=(kh == KH - 1),
                            )
                    nc.scalar.copy(
                        bu[:, ci * TC + half * 1024: ci * TC + (half + 1) * 1024],
                        ps[:])
            return bu

        # software pipeline: iteration i+1's loads + first Bu tile are
        # emitted before iteration i's projection stripes, so PE can start
        # the next iteration inside its post-exchange hole and DVE's first
        # pre-rotation of i+1 is not gated on the full PE drain of i.
        front = emit_loads(0)
        bu_next = emit_bu(front, 0)

        for _iter in range(iters):
            bin_d = bin_db[_iter % NB]
            bout_d = bout_db[_iter % NB]
            bin4_d = bin4_db[_iter % NB] if percc else None
            bout4_d = bout4_db[_iter % NB] if percc else None
            xT_sb = front["xT"]
            BT_sb = front["BT"]
            tabs1 = front["tabs"]
            cst_sb = front["cst"]

            # ---- phase 1: Bu matmuls, shared pre-rotations, scans ----
            v_sb = {}
            chv_sb = {}
            epk = p_sm.tile([128, 16], F32, tag="epk", name="ep")      # (nt, dir) -> packed (128, 2TC) f16 scan outputs
            for nt in range(NT):
                nsl = slice(nt * 128, (nt + 1) * 128)
                if nt in tabs1:
                    cos_t, sin_t = tabs1[nt]
                else:
                    cos_t = p_cos.tile([128, TC], F16, tag="cos", name="c1")
                    nc.sync.dma_start(cos_t[:], cos_d[nsl, :])
                    sin_t = p_sin.tile([128, TC], F16, tag="sin", name="s1")
                    nc.sync.dma_start(sin_t[:], sin_d[nsl, :])
                    tabs1[nt] = (cos_t, sin_t)
                epk4 = (p_ch.tile([128, 4], F32, tag="ek4", name="e4")
                        if percc else None)
                bu = bu_next if nt == 0 else emit_bu(front, nt)
                bu3 = bu[:].rearrange("p (c t) -> p c t", c=2)
                bsw = bu3[:, ::-1, :]
                prod1 = p_wq.tile([128, 2 * TC], F16, tag="wq", name="p1")
                qr = p_wq.tile([128, 2 * TC], F16, tag="wq", name="qr")
                p13 = prod1[:].rearrange("p (c t) -> p c t", c=2)
                p2l = p_b4.tile([128, TC], F16, tag="b4", name="2l")
                p2h = p_b4.tile([128, TC], F16, tag="b4", name="2h")
                nc.vector.tensor_tensor(p13, _dup2(cos_t[:], TC), bu3, MUL)
                peng = nc.gpsimd if pre_pool else nc.vector
                peng.tensor_tensor(p2l[:], sin_t[:], bu[:, TC:2 * TC], MUL)
                peng.tensor_tensor(p2h[:], sin_t[:], bu[:, 0:TC], MUL)
                # qr = bwd scan input, stored time-reversed
                nc.vector.tensor_tensor(qr[:, 0:TC][:, ::-1],
                                        prod1[:, 0:TC], p2l[:], SUB)
                nc.vector.tensor_tensor(qr[:, TC:2 * TC][:, ::-1],
                                        prod1[:, TC:2 * TC], p2h[:], ADD)
                # wf in place over prod1
                nc.vector.tensor_tensor(prod1[:, 0:TC],
                                        prod1[:, 0:TC], p2l[:], ADD)
                nc.vector.tensor_tensor(prod1[:, TC:2 * TC],
                                        prod1[:, TC:2 * TC], p2h[:], SUB)
                wf = prod1
                rbc = cst_sb[nt][:, 0:1].broadcast_to([128, TC])
                for di_, (d_, src) in enumerate((("f", wf), ("b", qr))):
                    v = p_v.tile([128, 2 * TC], F16, tag="v", name="v")
                    nc.vector.tensor_tensor_scan(
                        v[:, 0:TC], rbc, src[:, 0:TC], 0.0, MUL, ADD)
                    nc.vector.tensor_tensor_scan(
                        v[:, TC:2 * TC], rbc, src[:, TC:2 * TC], 0.0, MUL, ADD)
                    v_sb[(nt, d_)] = v
                    if percc:
                        nc.scalar.copy(epk4[:, 2 * di_:2 * di_ + 2],
                                       v[:, TC - 1::TC])
                    else:
                        nc.scalar.copy(
                            epk[:, (0 if d_ == "f" else 8) + nt * 2:
                                (0 if d_ == "f" else 8) + nt * 2 + 2],
                            v[:, TC - 1::TC])
                if percc:
                    # fire this tile's exchange while later tiles still scan
                    nc.sync.dma_start(bin4_d[nt][:, :], epk4[:])
                    if profile or fake_cc:
                        for j in range(NCORES):
                            nc.sync.dma_start(bout4_d[nt].ap()[j, :, :],
                                              bin4_d[nt][:, :])
                    else:
                        nc.gpsimd.collective_compute(
                            "AllGather", mybir.AluOpType.bypass,
                            replica_groups=[list(range(NCORES))],
                            ins=[bin4_d[nt].ap().opt()],
                            outs=[bout4_d[nt].ap().opt()],
                        )

            if not percc:
                # ---- single carry exchange of all raw end states ----
                nc.sync.dma_start(bin_d[:, :], epk[:])
                if profile or fake_cc:
                    for j in range(NCORES):
                        nc.sync.dma_start(bout_d.ap()[j, :, :], bin_d[:, :])
                else:
                    nc.gpsimd.collective_compute(
                        "AllGather", mybir.AluOpType.bypass,
                        replica_groups=[list(range(NCORES))],
                        ins=[bin_d.ap().opt()], outs=[bout_d.ap().opt()],
                    )
            # one contiguous gather of all end states: 64B runs instead of
            # 16 strided DMAs of 8B runs (descriptor-count bound)
            egall = p_sm.tile([128, 128], F32, tag="ega", name="ga")
            if percc:
                for nt in range(NT):
                    nc.sync.dma_start(
                        egall[:, nt * 32:(nt + 1) * 32].rearrange("p (j q) -> p j q", j=NCORES),
                        bout4_d[nt].ap().rearrange("j p q -> p j q"))
            else:
                nc.sync.dma_start(
                    egall[:].rearrange("p (j q) -> p j q", j=NCORES),
                    bout_d.ap().rearrange("j p q -> p j q"))
            for nt in range(NT):
                nsl = slice(nt * 128, (nt + 1) * 128)
                w4_t = p_sm.tile([128, 64], F32, tag="w4", name="w4")
                nc.sync.dma_start(w4_t[:], W4_d[nsl, :])
                if percc:
                    e4 = egall[:, nt * 32:(nt + 1) * 32].rearrange(
                        "p (j g c) -> p j g c", j=NCORES, g=2)
                else:
                    e4 = egall[:].rearrange(
                        "p (j g x) -> p j g x", j=NCORES, g=2)[:, :, :, nt * 2:nt * 2 + 2]
                chv = p_ch.tile([128, 4], F32, tag="chv", name="ch")
                deng = nc.gpsimd if dots_pool else nc.vector
                for half, csl in ((0, slice(0, 4, 2)), (1, slice(1, 4, 2))):
                    w4v = w4_t[:, half * 32:(half + 1) * 32].rearrange(
                        "p (j g c) -> p j g c", j=NCORES, g=2)
                    pr = p_sm.tile([128, 32], F32, tag="pr", name="pr")
                    deng.tensor_tensor(
                        pr[:].rearrange("p (g j c) -> p j g c", g=2, j=NCORES),
                        w4v, e4, MUL)
                    nc.vector.tensor_reduce(
                        chv[:, csl].rearrange("p (s o) -> p s o", o=1),
                        pr[:].rearrange("p (s m) -> p s m", s=2),
                        mybir.AxisListType.X, ADD)
                chv_sb[nt] = chv

            # prefetch correction tables while phase 1 drains (rpw pool has
            # 2 bufs: nt2/3 allocations recycle nt0/1 after their products)
            rpw_sb = {}
            for nt in range(NT):
                rpw_t = p_rpw.tile([128, TC], F16, tag="rpw", name="rq")
                nc.sync.dma_start(rpw_t[:], rpw_d[nt * 128:(nt + 1) * 128, :])
                rpw_sb[nt] = rpw_t

            # ---- late loads (needed only from mid-kernel on) ----
            CT_sb = {}
            for key, dd in CT_d.items():
                for nt in range(NT):
                    t_ = p_CT.tile([128, H], F16, tag="CT", name="ct")
                    nc.sync.dma_start(t_[:], dd[nt * 128:(nt + 1) * 128, :])
                    CT_sb[key + (nt,)] = t_

            # ---- software-pipelined front of iteration i+1 ----
            if _iter + 1 < iters:
                front_n = emit_loads(_iter + 1)
                bu_n = emit_bu(front_n, 0)
            else:
                front_n = bu_n = None

            # ---- phase 2a: per nt full-width corrections ----
            for nt in range(NT):
                nsl = slice(nt * 128, (nt + 1) * 128)
                chv = chv_sb[nt]
                rpw_t = rpw_sb[nt]
                v = v_sb[(nt, "f")]
                vb = v_sb[(nt, "b")]
                for ci, (vv, sc) in enumerate(((v, 0), (v, 1), (vb, 2), (vb, 3))):
                    t_ = p_b4.tile([128, TC], F16, tag="b4", name="t4")
                    # product on DVE tensor_scalar (4x mode, 594ns) instead of
                    # Act (1893ns): avoids starving the corr adds on DVE
                    nc.vector.tensor_scalar(t_[:], rpw_t[:],
                                            chv[:, sc:sc + 1], None, MUL)
                    half = slice(0, TC) if ci % 2 == 0 else slice(TC, 2 * TC)
                    eng = nc.gpsimd if corr_pool else nc.vector
                    eng.tensor_tensor(vv[:, half], t_[:], vv[:, half], ADD)
            rpw_sb = {}

            # ---- phase 2b: lc-major striped post-rot + output projections ----
            sb_sb = {nt: p_wq.tile([128, 2 * TC], F16, tag="wq", name="sb")
                     for nt in range(NT)}
            for lc in range(NL):
                a, b = lc * LB, (lc + 1) * LB
                ra, rb = TC - b, TC - a
                lsl = slice(a, b)
                hsl_i = slice(TC + a, TC + b)
                rsl = slice(ra, rb)
                for nt in range(NT):
                    cos_t, sin_t = tabs1[nt]
                    v = v_sb[(nt, "f")]
                    vb = v_sb[(nt, "b")]
                    sb = sb_sb[nt]
                    v3 = v[:].rearrange("p (c t) -> p c t", c=2)
                    vsw = v3[:, ::-1, :]
                    vb3 = vb[:].rearrange("p (c t) -> p c t", c=2)
                    vbsw = vb3[:, ::-1, :]
                    pp = p_b4.tile([128, 2 * LB], F16, tag="b4", name="pp")
                    zz = p_b4.tile([128, 2 * LB], F16, tag="b4", name="zz")
                    pp3 = pp[:].rearrange("p (c t) -> p c t", c=2)
                    zz3 = zz[:].rearrange("p (c t) -> p c t", c=2)
                    nc.vector.tensor_tensor(
                        pp3, _dup2(cos_t[:], TC)[:, :, lsl], v3[:, :, lsl], MUL)
                    nc.vector.tensor_tensor(
                        zz3, _dup2(sin_t[:], TC)[:, :, lsl], vsw[:, :, lsl], MUL)
                    nc.vector.tensor_tensor(v[:, lsl], pp[:, 0:LB], zz[:, 0:LB], SUB)
                    nc.vector.tensor_tensor(v[:, hsl_i], pp[:, LB:2 * LB],
                                            zz[:, LB:2 * LB], ADD)
                    # bwd products on the (otherwise idle) Pool engine
                    ppb = p_b4.tile([128, 2 * LB], F16, tag="b4", name="pb")
                    zzb = p_b4.tile([128, 2 * LB], F16, tag="b4", name="zb")
                    ppb3 = ppb[:].rearrange("p (c t) -> p c t", c=2)
                    zzb3 = zzb[:].rearrange("p (c t) -> p c t", c=2)
                    nc.vector.tensor_tensor(
                        ppb3, _dup2(cos_t[:, ::-1], TC)[:, :, rsl], vb3[:, :, rsl], MUL)
                    nc.vector.tensor_tensor(
                        zzb3, _dup2(sin_t[:, ::-1], TC)[:, :, rsl], vbsw[:, :, rsl], MUL)
                    nc.vector.tensor_tensor(sb[:, lsl][:, ::-1],
                                            ppb[:, 0:LB], zzb[:, 0:LB], ADD)
                    nc.vector.tensor_tensor(sb[:, hsl_i][:, ::-1],
                                            ppb[:, LB:2 * LB], zzb[:, LB:2 * LB], SUB)
                for ht in range(HT):
                    hsl = slice(ht * 128, (ht + 1) * 128)
                    # FD=512 sub-chains: one matmul must fit one PSUM bank
                    for half in range(2):
                        aa = a + half * 512
                        l5 = slice(aa, aa + 512)
                        h5 = slice(TC + aa, TC + aa + 512)
                        ps = p_ops.tile([128, 512], F32, tag="ops", name="op")
                        groups = []
                        for nt in range(NT):
                            groups.append((CT_sb[("f", "r", nt)], v_sb[(nt, "f")], l5))
                            groups.append((CT_sb[("f", "i", nt)], v_sb[(nt, "f")], h5))
                            groups.append((CT_sb[("b", "r", nt)], sb_sb[nt], l5))
                            groups.append((CT_sb[("b", "i", nt)], sb_sb[nt], h5))
                        for gi, (ct, sv, sl_) in enumerate(groups):
                            nc.tensor.matmul(
                                ps[:], ct[:, hsl], sv[:, sl_],
                                start=(gi == 0), stop=(gi == len(groups) - 1),
                            )
                        yo = p_yo.tile([128, 512], F32, tag="yo", name="yo")
                        nc.scalar.copy(yo[:], ps[:])
                        nc.sync.dma_start(yT_d[hsl, l5], yo[:])

            front, bu_next = front_n, bu_n

    nc.compile()
    return nc


def _host_prep(x, theta_log, nu_log, B_re, B_im, C_re, C_im, C_re2, C_im2, D):
    f64 = np.float64
    theta = np.exp(theta_log.astype(f64))
    r = np.exp(-np.exp(nu_log.astype(f64)))
    gamma = np.sqrt(1.0 - r ** 2)
    Bn = (B_re.astype(f64) + 1j * B_im.astype(f64)) * gamma[:, None]
    Lam = r * np.exp(1j * theta)
    tau = np.arange(TC, dtype=f64)
    cosT = np.cos(theta[:, None] * tau).astype(np.float16)
    sinT = np.sin(theta[:, None] * tau).astype(np.float16)
    rpow = (r[:, None] ** (tau + 1)).astype(np.float16)
    # postfirst correction tables (correction applied in the OUTPUT frame,
    # after the local post-rotation):
    #   fwd:  s_f[m] += r^{m+1} ( cos(th m) chv_re - sin(th m) chv_im )
    #   bwd:  s_b[m] += r^{TC-m}( cos(th m) chv_re + sin(th m) chv_im )
    # (bwd written-frame index m = TC-1-k, scan correction r^{k+1},
    #  post-rot phase e^{-i th m}; the +i/-i sign difference between the
    #  two directions is absorbed into the W4 recovery columns, so both
    #  use the same (cos, sin)-combination signs here; tabB/tabD are the
    #  sin tables, negated host-side where needed at the W4 stage.)
    rf = r[:, None] ** (tau + 1)
    rb = r[:, None] ** (TC - tau)
    tabA = (rf * np.cos(theta[:, None] * tau)).astype(np.float16)
    tabB = (rf * np.sin(theta[:, None] * tau)).astype(np.float16)
    tabC = (rb * np.cos(theta[:, None] * tau)).astype(np.float16)
    tabD = (rb * np.sin(theta[:, None] * tau)).astype(np.float16)
    consts = np.zeros((N, 8), np.float32)
    consts[:, 0] = r
    xT = np.ascontiguousarray(x.T.astype(np.float16))        # (H, T)
    BTre = np.ascontiguousarray(Bn.real.T.astype(np.float16))
    BTim = np.ascontiguousarray(Bn.imag.T.astype(np.float16))
    C1 = C_re.astype(f64) + 1j * C_im.astype(f64)
    C2 = C_re2.astype(f64) + 1j * C_im2.astype(f64)
    CT = {
        ("f", "r"): C1.real.T, ("f", "i"): -C1.imag.T,
        ("b", "r"): C2.real.T, ("b", "i"): -C2.imag.T,
    }
    CT = {k: np.ascontiguousarray(v.astype(np.float16)) for k, v in CT.items()}
    LamTC = Lam ** TC
    phase = np.exp(1j * theta * TC)
    W4 = []
    for k in range(NCORES):
        wf = np.zeros((N, NCORES), np.complex128)
        wb = np.zeros((N, NCORES), np.complex128)
        for j in range(k):
            wf[:, j] = phase * LamTC ** (k - 1 - j)
        for j in range(k + 1, NCORES):
            wb[:, j] = phase * LamTC ** (j - k - 1)

        # layout matches the contiguous gather: element (j, g, c) of the
        # gathered end states pairs with W4re/W4im at flat index j*4+g*2+c
        w4re = np.zeros((N, NCORES, 2, 2), np.float64)
        w4im = np.zeros((N, NCORES, 2, 2), np.float64)
        w4re[:, :, 0, 0] = wf.real
        w4re[:, :, 0, 1] = -wf.imag
        w4re[:, :, 1, 0] = wb.real
        w4re[:, :, 1, 1] = -wb.imag
        w4im[:, :, 0, 0] = wf.imag
        w4im[:, :, 0, 1] = wf.real
        w4im[:, :, 1, 0] = wb.imag
        w4im[:, :, 1, 1] = wb.real
        w4 = np.concatenate(
            [w4re.reshape(N, 32), w4im.reshape(N, 32)], axis=1).astype(np.float32)
        W4.append(np.ascontiguousarray(w4))
    Dx = (D.astype(f64)[None, :] * x.astype(f64)).astype(np.float32)
    return xT, BTre, BTim, cosT, sinT, rpow, consts, CT, W4, Dx


def make_in_maps(inputs):
    xT, BTre, BTim, cosT, sinT, rpow, consts, CT, W4, Dx = _host_prep(**inputs)
    in_maps = []
    for k in range(NCORES):
        in_maps.append({
            "xT": np.ascontiguousarray(xT[:, k * TC:(k + 1) * TC]),
            "BTre": BTre, "BTim": BTim,
            "cosT": cosT, "sinT": sinT, "rpow": rpow, "consts": consts,
            "CTfr": CT[("f", "r")], "CTfi": CT[("f", "i")],
            "CTbr": CT[("b", "r")], "CTbi": CT[("b", "i")],
            "W4": W4[k],
        })
    return in_maps, Dx


# engine-assignment flags used for the graded build (see bench2.py).
# Pool offloads measured SLOWER on HW (Pool TT is ~3.6x DVE per op and
# sits on coupled critical chains): keep everything on DVE/Act.
FLAGS = dict()


def kernel(**inputs):
    if "nc" not in _CACHE:
        _CACHE["nc"] = _build_nc(**FLAGS)
    nc = _CACHE["nc"]
    in_maps, Dx = make_in_maps(inputs)
    res = run_bass_kernel_spmd(nc, in_maps, core_ids=list(range(NCORES)))
    yT = np.concatenate([res.results[k]["yT"] for k in range(NCORES)], axis=1)
    return (np.ascontiguousarray(yT.T) + Dx).astype(np.float32)

